# revision 1
# baseline (speedup 1.0000x reference)
"""LocalWindowAttention Trainium2 Bass kernel.

Full-input contract: kernel(**inputs) takes the unsharded tensors
(x:[8,192,224,224], Wq/Wk/Wv/Wo:[192,192], bq/bk/bv/bo:[192]) and
returns the full [8,192,224,224] output.  Internally: data-parallel
over batch across 8 NeuronCores (1 image per core), weights replicated.

Math notes (vs reference):
  - H=W=224 divide by ws=7, so the reference's reflect-pad is a no-op.
  - V-bias folded out: softmax rows sum to 1, so
    softmax(S) @ (Vraw + bv 1^T) = softmax(S) @ Vraw + bv, and
    bo_eff = Wo @ bv + bo is applied in the final conv instead.
  - no max-subtraction in softmax: scores/sqrt(C) are O(+-6) for this
    input distribution; exp stays in fp32 range and ratios are exact.

Layout: on-chip pixel order is window-major (w, r, cc) within a strip
of 7 image rows (32 windows); x is DMA'd raster and re-ordered by the
(otherwise idle) GPSIMD engine; the final-conv evacuation converts
back to raster so HBM I/O stays contiguous.
"""

import math
from contextlib import ExitStack

import numpy as np

import concourse.bacc as bacc
import concourse.bass as bass
import concourse.tile as tile
from concourse import mybir
from concourse.bass_utils import run_bass_kernel_spmd

F32 = mybir.dt.float32

B, C, H, W = 8, 192, 224, 224
WS = 7
NSTRIP = H // WS            # 32 strips (one window-row each)
SP = WS * W                 # 1568 pixels per strip
NW = W // WS                # 32 windows per strip
WP = WS * WS                # 49 pixels per window
NT = 392                    # N-tile = 8 windows
NGRP = SP // NT             # 4 groups per strip
C0, C1 = 128, 64            # channel chunks (192 = 128 + 64)
SCALE = 1.0 / math.sqrt(C)

_CACHE = {}


def _build():
    nc = bacc.Bacc(None, target_bir_lowering=False)

    x_d = nc.dram_tensor("x", [C, H, W], F32, kind="ExternalInput")
    y_d = nc.dram_tensor("y", [C, H, W], F32, kind="ExternalOutput")
    w_d = {
        n: nc.dram_tensor(n, [C, C], F32, kind="ExternalInput")
        for n in ("wqT", "wkT", "wvT", "woT")
    }
    b_d = {
        n: nc.dram_tensor(n, [C, 1], F32, kind="ExternalInput")
        for n in ("bq", "bk", "bo")
    }
    ident_d = nc.inline_tensor(np.eye(WP, dtype=np.float32), name="ident49")

    with tile.TileContext(nc) as tc, ExitStack() as ctx:
        const = ctx.enter_context(tc.tile_pool(name="const", bufs=1))

        wt = {}
        for n in ("wqT", "wkT", "wvT", "woT"):
            t0 = const.tile([C0, C], F32, tag=f"{n}0")
            t1 = const.tile([C1, C], F32, tag=f"{n}1")
            nc.sync.dma_start(t0[:], w_d[n][0:C0, :])
            nc.sync.dma_start(t1[:], w_d[n][C0:C, :])
            wt[n] = (t0, t1)
        bias = {}
        for n in ("bq", "bk", "bo"):
            t0 = const.tile([C0, 1], F32, tag=f"{n}0")
            t1 = const.tile([C1, 1], F32, tag=f"{n}1")
            nc.sync.dma_start(t0[:], b_d[n][0:C0, :])
            nc.sync.dma_start(t1[:], b_d[n][C0:C, :])
            bias[n] = (t0, t1)
        ident = const.tile([WP, WP], F32, tag="ident")
        nc.sync.dma_start(ident[:], ident_d[:, :])

        xp = ctx.enter_context(tc.tile_pool(name="xp", bufs=2))
        qp = ctx.enter_context(tc.tile_pool(name="qp", bufs=2))
        kp = ctx.enter_context(tc.tile_pool(name="kp", bufs=2))
        vtp = ctx.enter_context(tc.tile_pool(name="vtp", bufs=1))
        pp = ctx.enter_context(tc.tile_pool(name="pp", bufs=2))
        smp = ctx.enter_context(tc.tile_pool(name="smp", bufs=4))
        otp = ctx.enter_context(tc.tile_pool(name="otp", bufs=2))
        outp = ctx.enter_context(tc.tile_pool(name="outp", bufs=2))

        psb = ctx.enter_context(
            tc.tile_pool(name="psb", bufs=2, space=bass.MemorySpace.PSUM)
        )
        pss = ctx.enter_context(
            tc.tile_pool(name="pss", bufs=2, space=bass.MemorySpace.PSUM)
        )
        psvt = ctx.enter_context(
            tc.tile_pool(name="psvt", bufs=2, space=bass.MemorySpace.PSUM)
        )
        pssc = ctx.enter_context(
            tc.tile_pool(name="pssc", bufs=1, space=bass.MemorySpace.PSUM)
        )
        pst = ctx.enter_context(
            tc.tile_pool(name="pst", bufs=1, space=bass.MemorySpace.PSUM)
        )

        for s in range(NSTRIP):
            # ---- load x strip (raster), then window-major copy on gpsimd
            xs = (
                xp.tile([C0, WS, W], F32, tag="x0", name="x0t", bufs=1),
                xp.tile([C1, WS, W], F32, tag="x1", name="x1t", bufs=1),
            )
            nc.sync.dma_start(xs[0][:], x_d[0:C0, 7 * s : 7 * s + 7, :])
            nc.sync.dma_start(xs[1][:], x_d[C0:C, 7 * s : 7 * s + 7, :])
            xwm = (
                xp.tile([C0, SP], F32, tag="xw0", name="xw0t"),
                xp.tile([C1, SP], F32, tag="xw1", name="xw1t"),
            )
            for t, w in zip(xs, xwm):
                nc.gpsimd.tensor_copy(
                    w[:].rearrange("c (w r cc) -> c w r cc", r=WS, cc=WS),
                    t[:].rearrange("c r (w cc) -> c w r cc", cc=WS),
                )

            # ---- q, k convs -> [c-chunk, 1568] window-major sbuf
            def conv_qk(pool, wname, bname, tag, eng):
                out0 = pool.tile([C0, SP], F32, tag=f"{tag}0")
                out1 = pool.tile([C1, SP], F32, tag=f"{tag}1")
                for nt in range(NGRP):
                    sl = slice(NT * nt, NT * nt + NT)
                    for mi, (mo, msz, ot) in enumerate(
                        ((0, C0, out0), (C0, C1, out1))
                    ):
                        ps = (psb if mi == 0 else pss).tile(
                            [msz, NT], F32, tag="big" if mi == 0 else "small"
                        )
                        for ki in range(2):
                            nc.tensor.matmul(
                                ps[:],
                                wt[wname][ki][:, mo : mo + msz],
                                xwm[ki][:, sl],
                                start=(ki == 0),
                                stop=(ki == 1),
                            )
                        if eng == "act":
                            nc.scalar.activation(
                                ot[:, sl],
                                ps[:],
                                mybir.ActivationFunctionType.Identity,
                                bias=bias[bname][mi][:],
                            )
                        else:
                            nc.vector.tensor_scalar_add(
                                ot[:, sl], ps[:], bias[bname][mi][:]
                            )
                return out0, out1

            q = conv_qk(qp, "wqT", "bq", "q", "act")
            k = conv_qk(kp, "wkT", "bk", "k", "dve")

            # ---- Vt conv: window w -> [49, 192] slice of vt
            vt = vtp.tile([WP, NW, C], F32, tag="vt")
            for bk in range(NW // 2):  # 2 windows per PSUM bank
                ps = psvt.tile([WP, 2, C], F32, tag="vt")
                for wi in range(2):
                    w = 2 * bk + wi
                    for ki in range(2):
                        nc.tensor.matmul(
                            ps[:, wi],
                            xwm[ki][:, WP * w : WP * w + WP],
                            wt["wvT"][ki][:],
                            start=(ki == 0),
                            stop=(ki == 1),
                        )
                if bk % 2 == 0:
                    nc.vector.tensor_copy(vt[:, 2 * bk : 2 * bk + 2], ps[:])
                else:
                    nc.scalar.activation(
                        vt[:, 2 * bk : 2 * bk + 2],
                        ps[:],
                        mybir.ActivationFunctionType.Copy,
                    )

            praw = pp.tile([WP, SP], F32, tag="praw")
            pnrm = pp.tile([WP, SP], F32, tag="pnrm")
            ptr = pp.tile([WP, NW, WP], F32, tag="ptr")  # P^T per window
            ot0 = otp.tile([C0, SP], F32, tag="ot0", bufs=1)
            ot1 = otp.tile([C1, SP], F32, tag="ot1", bufs=1)

            for g in range(NGRP):
                gsl = slice(NT * g, NT * g + NT)
                # scores: 8 windows -> one PSUM bank [49, 8, 49]
                sc = pssc.tile([WP, 8, WP], F32, tag="sc")
                for wi in range(8):
                    w = 8 * g + wi
                    for ki in range(2):
                        nc.tensor.matmul(
                            sc[:, wi],
                            q[ki][:, WP * w : WP * w + WP],
                            k[ki][:, WP * w : WP * w + WP],
                            start=(ki == 0),
                            stop=(ki == 1),
                        )
                # exp(S/sqrt(C)) -> praw
                nc.scalar.activation(
                    praw[:, gsl],
                    sc[:].rearrange("q w e -> q (w e)"),
                    mybir.ActivationFunctionType.Exp,
                    scale=SCALE,
                )
                # softmax denominator + normalize (normalize on gpsimd)
                sums = smp.tile([WP, 8], F32, tag="sums")
                rec = smp.tile([WP, 8], F32, tag="rec")
                nc.vector.reduce_sum(
                    sums[:],
                    praw[:, gsl].rearrange("q (w e) -> q w e", e=WP),
                    axis=mybir.AxisListType.X,
                )
                nc.vector.reciprocal(rec[:], sums[:])
                nc.gpsimd.tensor_mul(
                    pnrm[:, gsl].rearrange("q (w e) -> q w e", e=WP),
                    praw[:, gsl].rearrange("q (w e) -> q w e", e=WP),
                    rec[:].broadcast_to([WP, 8, WP]),
                )
                # transpose each window's P -> P^T, batch 8 per bank
                tps = pst.tile([WP, 8, WP], F32, tag="t")
                for wi in range(8):
                    w = 8 * g + wi
                    nc.tensor.transpose(
                        tps[:, wi],
                        pnrm[:, WP * w : WP * w + WP],
                        ident[:],
                    )
                nc.scalar.activation(
                    ptr[:, 8 * g : 8 * g + 8],
                    tps[:],
                    mybir.ActivationFunctionType.Copy,
                )
                # PV: per window, Vt stationary -> O^T [c, 49] slices
                po0 = psb.tile([C0, 8, WP], F32, tag="big")
                po1 = pss.tile([C1, 8, WP], F32, tag="small")
                for wi in range(8):
                    w = 8 * g + wi
                    nc.tensor.matmul(
                        po0[:, wi],
                        vt[:, w, 0:C0],
                        ptr[:, w],
                        start=True,
                        stop=True,
                    )
                    nc.tensor.matmul(
                        po1[:, wi],
                        vt[:, w, C0:C],
                        ptr[:, w],
                        start=True,
                        stop=True,
                    )
                nc.scalar.activation(
                    ot0[:, gsl],
                    po0[:].rearrange("c w e -> c (w e)"),
                    mybir.ActivationFunctionType.Copy,
                )
                nc.vector.tensor_copy(
                    ot1[:, gsl], po1[:].rearrange("c w e -> c (w e)")
                )

            # ---- final conv + bias; evac converts window-major -> raster
            outs = (
                outp.tile([C0, WS, W], F32, tag="out0", name="out0t"),
                outp.tile([C1, WS, W], F32, tag="out1", name="out1t"),
            )
            for nt in range(NGRP):
                sl = slice(NT * nt, NT * nt + NT)
                for mi, (mo, msz) in enumerate(((0, C0), (C0, C1))):
                    ps = (psb if mi == 0 else pss).tile(
                        [msz, NT], F32, tag="big" if mi == 0 else "small"
                    )
                    for ki, ot in enumerate((ot0, ot1)):
                        nc.tensor.matmul(
                            ps[:],
                            wt["woT"][ki][:, mo : mo + msz],
                            ot[:, sl],
                            start=(ki == 0),
                            stop=(ki == 1),
                        )
                    nc.scalar.activation(
                        outs[mi][:]
                        .rearrange("c r (w cc) -> c w r cc", cc=WS)[
                            :, 8 * nt : 8 * nt + 8
                        ],
                        ps[:].rearrange("c (w r cc) -> c w r cc", r=WS, cc=WS),
                        mybir.ActivationFunctionType.Identity,
                        bias=bias["bo"][mi][:],
                    )
            nc.sync.dma_start(y_d[0:C0, 7 * s : 7 * s + 7, :], outs[0][:])
            nc.sync.dma_start(y_d[C0:C, 7 * s : 7 * s + 7, :], outs[1][:])

    nc.compile()
    return nc


def kernel(x, Wq, bq, Wk, bk, Wv, bv, Wo, bo):
    if "nc" not in _CACHE:
        _CACHE["nc"] = _build()
    nc = _CACHE["nc"]

    f32 = np.float32
    shared = {
        "wqT": np.ascontiguousarray(np.asarray(Wq, f32).T),
        "wkT": np.ascontiguousarray(np.asarray(Wk, f32).T),
        "wvT": np.ascontiguousarray(np.asarray(Wv, f32).T),
        "woT": np.ascontiguousarray(np.asarray(Wo, f32).T),
        "bq": np.ascontiguousarray(np.asarray(bq, f32).reshape(C, 1)),
        "bk": np.ascontiguousarray(np.asarray(bk, f32).reshape(C, 1)),
        "bo": np.ascontiguousarray(
            (np.asarray(Wo, f32) @ np.asarray(bv, f32) + np.asarray(bo, f32)).reshape(
                C, 1
            )
        ),
    }
    x = np.asarray(x, f32)
    in_maps = [{"x": np.ascontiguousarray(x[b]), **shared} for b in range(B)]
    res = run_bass_kernel_spmd(
        nc, in_maps, core_ids=list(range(B)), trace=TRACE
    )
    _CACHE["last_result"] = res
    return np.stack([r["y"] for r in res.results], axis=0)


TRACE = False



# revision 8
# speedup vs baseline: 3.3719x; 3.3719x over previous
"""LocalWindowAttention Trainium2 Bass kernel (bf16 rewrite).

Full-input contract: kernel(**inputs) takes the unsharded tensors
(x:[8,192,224,224], Wq/Wk/Wv/Wo:[192,192], bq/bk/bv/bo:[192]) and
returns the full [8,192,224,224] output.  Data-parallel over batch
across 8 NeuronCores (1 image per core), weights replicated.

Math notes (vs reference):
  - H=W=224 divide by ws=7, so the reference's reflect-pad is a no-op.
  - Wo is folded into Wv: out_w = Wo (V_w P_w^T) + bo
      = (Wo Wv) x_w P_w^T + (Wo bv + bo)  (softmax rows sum to 1), so a
    single "Z conv" with Wov = Wo@Wv produces Z^T per window and the PV
    matmul directly yields the final output (no separate output conv).
  - all matmuls run in bf16 (4x faster row streaming than fp32 on the
    PE); scores accumulate in fp32 PSUM, softmax sums in fp32.
  - no max-subtraction in softmax: scores/sqrt(C) are O(+-6), exp stays
    in fp32 range.

Layout: x stays raster in SBUF; window views are strided APs.
q/k live as [C-chunk, 1568] window-major (conv PSUM evacs write them
window-major); scores are computed per window PAIR as [98, 98] with
garbage off-diagonal blocks (never read).  Z^T (=vt) is [49, 32, 192]
via per-window layout-B convs.  P^T via PE transposes, batched 2
windows per instruction.  PV output [C-chunk, 8, 49] is evac'd with
bias straight to the raster output tile.
"""

import math

import numpy as np
import ml_dtypes

import concourse.bacc as bacc
import concourse.bass as bass
import concourse.tile as tile
from concourse import mybir
from concourse.bass_utils import run_bass_kernel_spmd

F32 = mybir.dt.float32
BF16 = mybir.dt.bfloat16

B, C, H, W = 8, 192, 224, 224
WS = 7
NSTRIP = H // WS            # 32 strips (one window-row each)
SP = WS * W                 # 1568 pixels per strip
NW = W // WS                # 32 windows per strip
WP = WS * WS                # 49 pixels per window
C0, C1 = 128, 64            # channel chunks (192 = 128 + 64)
NGRP = 4                    # conv groups per strip (8 windows each)
NT = 392                    # pixels per conv group
NBLK = 4                    # scores blocks per strip
NPB = 4                     # window pairs per scores block
WPITCH = 64                 # q/k window pitch (49 cols used, 64-aligned)
QSP = NW * WPITCH           # 2048 cols per strip in q/k tiles
SCALE = 1.0 / math.sqrt(C)

_CACHE = {}
TRACE = False


def _build():
    nc = bacc.Bacc(None, target_bir_lowering=False)

    x_d = nc.dram_tensor("x", [C, H, W], F32, kind="ExternalInput")
    y_d = nc.dram_tensor("y", [C, H, W], F32, kind="ExternalOutput")
    wqk_d = nc.dram_tensor("wqkT", [C, 384], BF16, kind="ExternalInput")
    wov_d = nc.dram_tensor("wovT", [C, C], BF16, kind="ExternalInput")
    bias_d = nc.dram_tensor("biases", [C0, 6], F32, kind="ExternalInput")
    id_d = nc.dram_tensor("ident128", [128, 128], BF16, kind="ExternalInput")

    with tile.TileContext(nc) as tc:
        with tc.tile_pool(name="const", bufs=1) as const, \
             tc.tile_pool(name="xp", bufs=1) as xp, \
             tc.tile_pool(name="qkp", bufs=1) as qkp, \
             tc.tile_pool(name="smp", bufs=1) as smp, \
             tc.tile_pool(name="vtp", bufs=1) as vtp, \
             tc.tile_pool(name="outp", bufs=1) as outp, \
             tc.tile_pool(name="qkps", bufs=2, space=bass.MemorySpace.PSUM) as qkps, \
             tc.tile_pool(name="scps", bufs=1, space=bass.MemorySpace.PSUM) as scps, \
             tc.tile_pool(name="vcps", bufs=2, space=bass.MemorySpace.PSUM) as vcps, \
             tc.tile_pool(name="ptps", bufs=1, space=bass.MemorySpace.PSUM) as ptps, \
             tc.tile_pool(name="pops", bufs=1, space=bass.MemorySpace.PSUM) as pops:

            # ---- constants
            w0 = const.tile([C0, 384], BF16, tag="w0")
            w1 = const.tile([C1, 384], BF16, tag="w1")
            nc.sync.dma_start(w0[:], wqk_d[0:C0, :])
            nc.sync.dma_start(w1[:], wqk_d[C0:C, :])
            wv0 = const.tile([C0, C], BF16, tag="wv0")
            wv1 = const.tile([C1, C], BF16, tag="wv1")
            nc.sync.dma_start(wv0[:], wov_d[0:C0, :])
            nc.sync.dma_start(wv1[:], wov_d[C0:C, :])
            ball = const.tile([C0, 6], F32, tag="ball")
            nc.sync.dma_start(ball[:], bias_d[:, :])
            ident = const.tile([C0, C0], BF16, tag="ident")
            nc.sync.dma_start(ident[:], id_d[:, :])

            for s in range(NSTRIP):
                # ---- load x strip (raster), cast to bf16
                xf0 = xp.tile([C0, WS, W], F32, tag="xf0", name="xf0t", bufs=2)
                xf1 = xp.tile([C1, WS, W], F32, tag="xf1", name="xf1t", bufs=2)
                nc.sync.dma_start(xf0[:], x_d[0:C0, 7 * s : 7 * s + 7, :])
                nc.sync.dma_start(xf1[:], x_d[C0:C, 7 * s : 7 * s + 7, :])
                # cast to bf16 + reorder raster -> window-major in one copy
                xb0 = xp.tile([C0, SP], BF16, tag="xb0", name="xb0t", bufs=2)
                xb1 = xp.tile([C1, SP], BF16, tag="xb1", name="xb1t", bufs=2)
                nc.gpsimd.tensor_copy(
                    xb0[:].rearrange("c (w r cc) -> c w r cc", r=WS, cc=WS),
                    xf0[:].rearrange("c r (w cc) -> c w r cc", cc=WS),
                )
                nc.gpsimd.tensor_copy(
                    xb1[:].rearrange("c (w r cc) -> c w r cc", r=WS, cc=WS),
                    xf1[:].rearrange("c r (w cc) -> c w r cc", cc=WS),
                )

                # ---- joint q/k conv -> window-major bf16 with bias
                q0 = qkp.tile([C0, NW, WPITCH], BF16, tag="q0", name="q0t", bufs=2)
                k0 = qkp.tile([C0, NW, WPITCH], BF16, tag="k0", name="k0t", bufs=2)
                qh = qkp.tile([C1, NW, WPITCH], BF16, tag="qh", name="qht", bufs=2)
                kh = qkp.tile([C1, NW, WPITCH], BF16, tag="kh", name="kht", bufs=2)
                chunks = (
                    (0, C0, q0, 0, "act"),
                    (128, C0, k0, 1, "dve"),
                    (256, C1, qh, 2, "act"),
                    (320, C1, kh, 3, "dve"),
                )
                for g in range(NGRP):
                    mv0 = xb0[:, NT * g : NT * g + NT]
                    mv1 = xb1[:, NT * g : NT * g + NT]
                    sl = slice(NT * g, NT * g + NT)
                    for mo, msz, dst, bcol, eng in chunks:
                        ps = qkps.tile([C0, NT], F32, tag="qk", name="qkpst")
                        nc.tensor.matmul(
                            ps[0:msz], w0[:, mo : mo + msz], mv0,
                            start=True, stop=False,
                        )
                        nc.tensor.matmul(
                            ps[0:msz], w1[:, mo : mo + msz], mv1,
                            start=False, stop=True,
                        )
                        dv = dst[:, 8 * g : 8 * g + 8, 0:WP]
                        pv = ps[0:msz].rearrange("c (w e) -> c w e", e=WP)
                        if eng == "act":
                            nc.scalar.activation(
                                dv, pv,
                                mybir.ActivationFunctionType.Identity,
                                bias=ball[0:msz, bcol : bcol + 1],
                            )
                        else:
                            nc.vector.tensor_scalar_add(
                                dv, pv, ball[0:msz, bcol : bcol + 1],
                            )

                # ---- per-strip softmax tiles
                praw = smp.tile([C0, 16, WP], BF16, tag="praw", name="prawt", bufs=2)
                pnrm = smp.tile([C0, 16, WP], BF16, tag="pnrm", name="pnrmt", bufs=2)
                sums = smp.tile([C0, 16], F32, tag="sums", name="sumst", bufs=2)
                rec = smp.tile([C0, 16], F32, tag="rec", name="rect", bufs=2)
                rec16 = smp.tile([C0, 16], BF16, tag="rec16", name="rec16t", bufs=2)
                ptr = smp.tile([WP, 16, C0], BF16, tag="ptr", name="ptrt", bufs=2)
                vt = vtp.tile([WP, NW, C], BF16, tag="vt", name="vtt", bufs=2)

                vt_eng = ("act", "dve", "act", "dve")
                for b in range(NBLK):
                    # scores for 4 window pairs -> [98, 4, 98] PSUM
                    sps = scps.tile([C0, NPB, C0], F32, tag="sc", name="scpst")
                    for p4 in range(NPB):
                        p = NPB * b + p4
                        sl = slice(2 * p, 2 * p + 2)
                        nc.tensor.matmul(
                            sps[:, p4],
                            q0[:, sl].rearrange("c w e -> c (w e)"),
                            k0[:, sl].rearrange("c w e -> c (w e)"),
                            start=True, stop=False,
                        )
                        nc.tensor.matmul(
                            sps[:, p4],
                            qh[:, sl].rearrange("c w e -> c (w e)"),
                            kh[:, sl].rearrange("c w e -> c (w e)"),
                            start=False, stop=True,
                        )
                    # exp of the two diagonal 49x49 blocks of each pair
                    bsl = slice(NPB * b, NPB * b + NPB)
                    nc.scalar.activation(
                        praw[0:WP, bsl, :], sps[0:WP, :, 0:WP],
                        mybir.ActivationFunctionType.Exp, scale=SCALE,
                    )
                    nc.scalar.activation(
                        praw[64 : 64 + WP, bsl, :], sps[64 : 64 + WP, :, 64 : 64 + WP],
                        mybir.ActivationFunctionType.Exp, scale=SCALE,
                    )
                    # v(=Z) conv for this block's 8 windows, layout B:
                    # out[49, 192] = x_w^T @ WovT  (overlaps softmax chain)
                    for wpair in range(4):
                        wb = 8 * b + 2 * wpair
                        vps = vcps.tile([WP, 2, C], F32, tag="v", name="vpst")
                        for hh in range(2):
                            w = wb + hh
                            wsl = slice(WP * w, WP * w + WP)
                            nc.tensor.matmul(
                                vps[:, hh], xb0[:, wsl], wv0[:],
                                start=True, stop=False,
                            )
                            nc.tensor.matmul(
                                vps[:, hh], xb1[:, wsl], wv1[:],
                                start=False, stop=True,
                            )
                        eng = vt_eng[wpair]
                        dst = vt[:, wb : wb + 2, :]
                        if eng == "act":
                            nc.scalar.activation(
                                dst, vps[:],
                                mybir.ActivationFunctionType.Copy,
                            )
                        else:
                            nc.vector.tensor_copy(dst, vps[:])
                    # softmax tail: row sums, reciprocal, normalize
                    nc.vector.reduce_sum(
                        sums[:, bsl], praw[:, bsl, :],
                        axis=mybir.AxisListType.X,
                    )
                    nc.vector.reciprocal(rec[:, bsl], sums[:, bsl])
                    nc.vector.tensor_copy(rec16[:, bsl], rec[:, bsl])
                    nc.gpsimd.tensor_mul(
                        pnrm[:, bsl, :], praw[:, bsl, :],
                        rec16[:, bsl].broadcast_to([C0, NPB, WP]),
                    )
                    # P^T: transpose [98, 49] -> [49, 98] per pair
                    tps = ptps.tile([WP, NPB, C0], BF16, tag="t", name="tpst")
                    for p4 in range(NPB):
                        nc.tensor.transpose(
                            tps[:, p4], pnrm[:, NPB * b + p4, :], ident[:]
                        )
                    nc.vector.tensor_copy(ptr[:, bsl, :], tps[:])

                # ---- PV (= final output): out_w = Z_w P_w^T + bo_eff
                outs0 = outp.tile([C0, WS, W], F32, tag="o0", name="o0t", bufs=2)
                outs1 = outp.tile([C1, WS, W], F32, tag="o1", name="o1t", bufs=2)
                ov0 = outs0[:].rearrange("c r (w cc) -> c w r cc", cc=WS)
                ov1 = outs1[:].rearrange("c r (w cc) -> c w r cc", cc=WS)
                for g in range(NGRP):
                    po0 = pops.tile([C0, 8, WP], F32, tag="po0", name="po0t")
                    po1 = pops.tile([C1, 8, WP], F32, tag="po1", name="po1t")
                    for wi in range(8):
                        wab = 8 * g + wi
                        p, hh = wab // 2, wab % 2
                        mv = ptr[:, p, 64 * hh : 64 * hh + WP]
                        nc.tensor.matmul(
                            po0[:, wi], vt[:, wab, 0:C0], mv,
                            start=True, stop=True,
                        )
                        nc.tensor.matmul(
                            po1[:, wi], vt[:, wab, C0:C], mv,
                            start=True, stop=True,
                        )
                    nc.scalar.activation(
                        ov0[:, 8 * g : 8 * g + 8],
                        po0[:].rearrange("c w (r cc) -> c w r cc", cc=WS),
                        mybir.ActivationFunctionType.Identity,
                        bias=ball[0:C0, 4:5],
                    )
                    nc.vector.tensor_scalar_add(
                        ov1[:, 8 * g : 8 * g + 8],
                        po1[:].rearrange("c w (r cc) -> c w r cc", cc=WS),
                        ball[0:C1, 5:6],
                    )
                nc.sync.dma_start(y_d[0:C0, 7 * s : 7 * s + 7, :], outs0[:])
                nc.sync.dma_start(y_d[C0:C, 7 * s : 7 * s + 7, :], outs1[:])

    nc.compile()
    return nc


def kernel(x, Wq, bq, Wk, bk, Wv, bv, Wo, bo):
    if "nc" not in _CACHE:
        _CACHE["nc"] = _build()
    nc = _CACHE["nc"]

    f32 = np.float32
    bf16 = ml_dtypes.bfloat16
    Wq = np.asarray(Wq, f32)
    Wk = np.asarray(Wk, f32)
    Wv = np.asarray(Wv, f32)
    Wo = np.asarray(Wo, f32)
    bq = np.asarray(bq, f32)
    bk = np.asarray(bk, f32)
    bv = np.asarray(bv, f32)
    bo = np.asarray(bo, f32)

    wqkT = np.concatenate(
        [Wq.T[:, 0:C0], Wk.T[:, 0:C0], Wq.T[:, C0:C], Wk.T[:, C0:C]], axis=1
    )
    wovT = (Wo @ Wv).T
    bo_eff = Wo @ bv + bo
    biases = np.zeros((C0, 6), f32)
    biases[:, 0] = bq[0:C0]
    biases[:, 1] = bk[0:C0]
    biases[0:C1, 2] = bq[C0:C]
    biases[0:C1, 3] = bk[C0:C]
    biases[:, 4] = bo_eff[0:C0]
    biases[0:C1, 5] = bo_eff[C0:C]

    shared = {
        "wqkT": np.ascontiguousarray(wqkT.astype(bf16)),
        "wovT": np.ascontiguousarray(wovT.astype(bf16)),
        "biases": biases,
        "ident128": np.eye(128, dtype=bf16),
    }
    x = np.asarray(x, f32)
    in_maps = [{"x": np.ascontiguousarray(x[b]), **shared} for b in range(B)]
    res = run_bass_kernel_spmd(nc, in_maps, core_ids=list(range(B)), trace=TRACE)
    _CACHE["last_result"] = res
    return np.stack([r["y"] for r in res.results], axis=0)
